# revision 1
# baseline (speedup 1.0000x reference)
"""JSD loss kernel v8 (= v3 with stt products written to a scratch tile
instead of in-place over ep/eq, probing for an in-place DVE penalty) for Trainium2 (8 NeuronCores, SPMD data-parallel).

Math (see kernel.py): per token four vocab reductions:
  sp = sum exp(p)   sq = sum exp(q)   ap = sum exp(p)*(p-q)   aq = sum exp(q)*(p-q)
  kl_p + kl_q = ap/sp - aq/sq;  loss = 0.25 * sum(kl*mask) / count

v3 = v2 (flat contiguous chunks + f32->bf16 cast-during-DMA + all-bf16
compute) with the DVE stream cut from 5 to 3 instructions per chunk:
measured DVE runs ~0.8 ns/elem regardless of dtype (the cost model's
2x/4x bf16 modes do not materialize), so instruction COUNT is what
matters.  scalar_tensor_tensor fuses product+reduction in one op:
  sub:    df = pt - qt
  stt-ap: ep = (ep*1)*df  (in place), accum_out -> ap column
  stt-aq: eq = (eq*1)*df  (in place), accum_out -> aq column
Engine busy/pass (measured rates): DMA ~271us, DVE ~307us, ACT ~150us.
"""

import numpy as np

import concourse.bass as bass
import concourse.mybir as mybir
from concourse.bass_utils import run_bass_kernel_spmd

N_CORES = 8
B, S, V = 2, 2048, 32000
TOKENS = B * S            # 4096
TPC = TOKENS // N_CORES   # 512 tokens per core
P = 128                   # SBUF partitions
F = 8000                  # flat chunk free size (divides V)
RPT = V // F              # 4 partition-rows per token
ROWS = TPC * V // F       # 2048 dram rows
NITER = ROWS // P         # 16 chunks per pass
NBUF = 2                  # double buffering

ACT_PER = 2               # ACT ops per chunk
DVE_PER = 3               # DVE ops per chunk

_NC_CACHE = {}


def _build_nc(repeat: int = 1):
    f32 = mybir.dt.float32
    bf16 = mybir.dt.bfloat16
    Exp = mybir.ActivationFunctionType.Exp
    Alu = mybir.AluOpType

    nc = bass.Bass()
    p = nc.dram_tensor("p", [ROWS, F], f32, kind="ExternalInput")
    q = nc.dram_tensor("q", [ROWS, F], f32, kind="ExternalInput")
    # stats cols: [sp | sq | ap | aq] blocks of NITER
    out = nc.dram_tensor("out", [P, 4 * NITER], f32, kind="ExternalOutput")

    with (
        nc.sbuf_tensor([P, NBUF * F], bf16) as pt,
        nc.sbuf_tensor([P, NBUF * F], bf16) as qt,
        nc.sbuf_tensor([P, NBUF * F], bf16) as ep,
        nc.sbuf_tensor([P, NBUF * F], bf16) as eq,
        nc.sbuf_tensor([P, F], bf16) as df,
        nc.sbuf_tensor([P, F], bf16) as pr,
        nc.sbuf_tensor([P, 4 * NITER], f32) as stats,
        nc.semaphore("dma_s") as dma_s,
        nc.semaphore("act_sem") as act_sem,
        nc.semaphore("dve_sem") as dve_sem,
        nc.semaphore("out_sem") as out_sem,
        nc.Block() as block,
    ):
        NTOT = NITER * repeat

        def src(tensor, i):
            c = i % NITER
            return tensor[c * P : (c + 1) * P, :]

        def slot(tile, i):
            s = i % NBUF
            return tile[:, s * F : (s + 1) * F]

        def col(base, i):
            c = base * NITER + i % NITER
            return stats[:, c : c + 1]

        @block.gpsimd
        def _(gpsimd):
            for i in range(NTOT):
                if i >= NBUF:
                    j = i - NBUF
                    # pt/qt slots free once chunk j's exps and sub have read them
                    gpsimd.wait_ge(act_sem, j * ACT_PER + 2)
                    gpsimd.wait_ge(dve_sem, j * DVE_PER + 1)
                gpsimd.dma_start(out=slot(pt, i), in_=src(p, i)).then_inc(dma_s, 16)
                gpsimd.dma_start(out=slot(qt, i), in_=src(q, i)).then_inc(dma_s, 16)

        @block.scalar
        def _(scalar):
            for i in range(NTOT):
                scalar.wait_ge(dma_s, i * 32 + 16)
                if i >= NBUF:
                    # ep slot busy until chunk i-NBUF's stt-ap has read it
                    scalar.wait_ge(dve_sem, (i - NBUF) * DVE_PER + 2)
                nc.scalar.activation(
                    slot(ep, i), slot(pt, i), Exp, accum_out=col(0, i)
                ).then_inc(act_sem, 1)
                scalar.wait_ge(dma_s, i * 32 + 32)
                if i >= NBUF:
                    scalar.wait_ge(dve_sem, (i - NBUF) * DVE_PER + 3)
                nc.scalar.activation(
                    slot(eq, i), slot(qt, i), Exp, accum_out=col(1, i)
                ).then_inc(act_sem, 1)

        @block.vector
        def _(vector):
            for i in range(NTOT):
                vector.wait_ge(dma_s, i * 32 + 32)
                nc.vector.tensor_sub(df[:], slot(pt, i), slot(qt, i)).then_inc(
                    dve_sem, 1
                )
                vector.wait_ge(act_sem, i * ACT_PER + 1)
                nc.vector.scalar_tensor_tensor(
                    pr[:], slot(ep, i), 1.0, df[:], Alu.mult, Alu.mult,
                    accum_out=col(2, i),
                ).then_inc(dve_sem, 1)
                vector.wait_ge(act_sem, i * ACT_PER + 2)
                nc.vector.scalar_tensor_tensor(
                    pr[:], slot(eq, i), 1.0, df[:], Alu.mult, Alu.mult,
                    accum_out=col(3, i),
                ).then_inc(dve_sem, 1)

        @block.sync
        def _(sync):
            sync.wait_ge(act_sem, NTOT * ACT_PER)
            sync.wait_ge(dve_sem, NTOT * DVE_PER)
            sync.dma_start(out=out[:, :], in_=stats[:, :]).then_inc(out_sem, 16)
            sync.wait_ge(out_sem, 16)

    return nc


def get_nc(repeat: int = 1):
    if repeat not in _NC_CACHE:
        _NC_CACHE[repeat] = _build_nc(repeat)
    return _NC_CACHE[repeat]


def make_in_maps(p, q):
    p2 = np.ascontiguousarray(np.asarray(p, dtype=np.float32).reshape(TOKENS, V))
    q2 = np.ascontiguousarray(np.asarray(q, dtype=np.float32).reshape(TOKENS, V))
    return [
        {
            "p": p2[k * TPC : (k + 1) * TPC].reshape(ROWS, F),
            "q": q2[k * TPC : (k + 1) * TPC].reshape(ROWS, F),
        }
        for k in range(N_CORES)
    ]


def finish_on_host(results, mask):
    """results: per-core dicts with 'out' [P, 4*NITER]; returns f32 scalar."""

    def tok(o, base):
        # stats[j, base*NITER + c] holds the partial of flat row g = c*P + j
        a = o[:, base * NITER : (base + 1) * NITER]  # [P, NITER]
        return a.T.reshape(TPC, RPT).sum(axis=1)     # row-major g -> tokens

    kl_all = []
    for r in results:
        o = np.asarray(r["out"], dtype=np.float64)
        sp, sq, ap, aq = (tok(o, b) for b in range(4))
        kl_all.append(ap / sp - aq / sq)
    kl = np.concatenate(kl_all)
    w = np.asarray(mask).reshape(-1).astype(np.float64)
    n = max(w.sum(), 1.0)
    loss = 0.25 * float((kl * w).sum()) / n
    return np.float32(loss)


def kernel(p, q, mask):
    nc = get_nc()
    res = run_bass_kernel_spmd(nc, make_in_maps(p, q), list(range(N_CORES)))
    return finish_on_host(res.results, mask)

